# revision 30
# baseline (speedup 1.0000x reference)
"""Multi-head self-attention (B=8, N=1024, C=768, H=12, D=64) on 8 Trainium2
NeuronCores, batch-parallel (one batch element per core).

v2: fully software-pipelined single-pass schedule. The exp stream (ACT) starts
as soon as Q/K for head-pair 0 exist (~10us in) and the V/QK/proj matmuls are
interleaved into the attention windows as PE filler work, instead of running
as serial phases with ACT idle.

Per-core dataflow (activations feature-major, "T" = [feature, token]):
  xT [768,1024] --(PE)--> QT,KT [768,1024] (d-major) and V [1024,768+ones]
  S^T[k,q] = KT_h^T x QT_h            (K=d=64; two heads of a pair via PE
                                       row-tiling at partitions 0/64,
                                       emitted back-to-back so the two
                                       halves stream concurrently)
  E = exp(S^T * scale) -> fp16        (ACT; no max-subtract: |S*scale| < 9)
  ctxU^T[d,q] (+den row) = V_ext_h^T x E   (ones column in V -> softmax den)
  normalize: recip(den) from PSUM, K=1 masked ones-matmul broadcast, one mul
  out[q,o] = ctxN^T x wpT + bias(bcast, DVE add)

Schedule: pair p's S/exp windows carry PV of pair p-1 (one-pair lag) plus a
filler unit per kt (remaining QK jt-slices, V token-tiles). All PSUM evacs run
on DVE; ACT does exp only. V-ones + matmul masks are built on-chip (memset)
instead of the 12k-packet strided DMA flood the old version had.
"""
import numpy as np

import concourse.bass as bass
import concourse.tile as tile
from concourse import bacc, mybir
from concourse.bass_utils import run_bass_kernel_spmd

N_CORES = 8
N = 1024          # tokens per core (batch element)
C = 768           # model dim
H = 12            # heads
D = 64            # head dim
SCALE = D ** -0.5
NT = N // 128     # 8 token tiles
CT = C // 128     # 6 feature tiles
F32 = mybir.dt.float32
F32R = mybir.dt.float32r
FP16 = mybir.dt.float16
EXP = mybir.ActivationFunctionType.Exp


def _r(ap):
    return ap.bitcast(F32R)


def build():
    nc = bacc.Bacc(
        "TRN2", target_bir_lowering=False, debug=False, num_devices=N_CORES
    )
    xT_d = nc.dram_tensor("xT", [C, N], FP16, kind="ExternalInput").ap()
    wqk_d = nc.dram_tensor("wqk", [12 * 128, C], FP16, kind="ExternalInput").ap()
    wv_d = nc.dram_tensor("wv", [C, C], FP16, kind="ExternalInput").ap()
    wp_d = nc.dram_tensor("wp", [C, C], FP16, kind="ExternalInput").ap()
    bias_d = nc.dram_tensor("bias_bc", [128, C], F32, kind="ExternalInput").ap()
    ones_d = nc.dram_tensor("ones_v", [128, H], FP16, kind="ExternalInput").ap()
    mask_d = nc.dram_tensor("ones_mask", [2, 128], FP16, kind="ExternalInput").ap()
    out_d = nc.dram_tensor("out", [N, C], F32, kind="ExternalOutput").ap()

    with tile.TileContext(nc) as tc:
        with (
            tc.tile_pool(name="big", bufs=1) as big,
            tc.tile_pool(name="e", bufs=18) as ep,
            tc.tile_pool(name="outb", bufs=2) as outp,
            tc.tile_pool(name="norm", bufs=4) as normp,
            tc.tile_pool(name="psA", bufs=2, space="PSUM") as psA,
            tc.tile_pool(name="psC", bufs=2, space="PSUM") as psC,
        ):
            # ---- persistent SBUF tensors -------------------------------
            xqk = big.tile([128, CT, N], FP16, name="xqk", tag="xqk")
            wqk = big.tile([128, 12, C], FP16, name="wqk", tag="wqk")
            wvs = big.tile([128, CT, C], FP16, name="wvs", tag="wvs")
            wps = big.tile([128, CT, C], FP16, name="wps", tag="wps")
            QT = big.tile([128, CT, N], FP16, name="QT", tag="QT")
            KT = big.tile([128, CT, N], FP16, name="KT", tag="KT")
            V = big.tile([128, NT, H * (D + 1)], FP16, name="V", tag="V")
            ctxN = big.tile([128, CT, N], FP16, name="ctxN", tag="ctxN")
            bias_sb = big.tile([128, C], F32, name="bias_sb", tag="bias")
            ones_sb = big.tile([128, H], FP16, name="ones_sb", tag="ones")
            ones_mask = [
                big.tile([1, 128], FP16, name=f"mask{i}", tag=f"mask{i}")
                for i in range(2)
            ]

            # ---- input DMAs (order = earliest-needed first) ------------
            # sync queue: per-ct (x, wqk) pairs so the head-pair-0 QK
            # projection streams behind the DMAs ct by ct; all contiguous
            for ct in range(CT):
                nc.sync.dma_start(xqk[:, ct, :], xT_d[ct * 128:(ct + 1) * 128, :])
            for b in range(12):
                nc.sync.dma_start(wqk[:, b, :], wqk_d[b * 128:(b + 1) * 128, :])
            # gpsimd queue: V weights, proj weights, constants (contiguous)
            nc.gpsimd.dma_start(ones_sb[:], ones_d[:])
            for i in range(2):
                nc.gpsimd.dma_start(ones_mask[i][:], mask_d[i:i + 1, :])
            for ct in range(CT):
                nc.gpsimd.dma_start(wvs[:, ct, :], wv_d[ct * 128:(ct + 1) * 128, :])
            for ct in range(CT):
                nc.gpsimd.dma_start(wps[:, ct, :], wp_d[ct * 128:(ct + 1) * 128, :])
            nc.gpsimd.dma_start(bias_sb[:], bias_d[:])

            # scatter the ones column into V on-chip (plain DVE copies)
            v4 = V[:].rearrange("p nt (h e) -> p nt h e", e=D + 1)
            for nt in range(NT):
                nc.vector.tensor_copy(
                    v4[:, nt, :, D:D + 1],
                    ones_sb[:].rearrange("p (h o) -> p h o", o=1),
                )


            # ---- work units --------------------------------------------
            def c_unit(jt, base):
                """QK projection slice jt (features jt*128..+127) for Q
                (base=0) or K (base=1); writes QT/KT[:, jt, :]."""
                ps = psA.tile([128, N], F32, tag="ps", name=f"c{base}_{jt}")
                for ct in range(CT):
                    lhsT = wqk[:, 2 * jt + base, ct * 128:(ct + 1) * 128]
                    for qc in range(2):
                        nc.tensor.matmul(
                            ps[:, qc * 512:(qc + 1) * 512],
                            lhsT,
                            xqk[:, ct, qc * 512:(qc + 1) * 512],
                            start=(ct == 0),
                            stop=(ct == CT - 1),
                        )
                dst = QT if base == 0 else KT
                nc.vector.tensor_copy(dst[:, jt, :], ps[:])

            def b_unit(nt):
                """V projection for token tile nt; writes V[:, nt, :]."""
                pv = psA.tile([128, N], F32, tag="ps", name=f"pv{nt}")
                for ct in range(CT):
                    lhsT = xqk[:, ct, nt * 128:(nt + 1) * 128]
                    for lo, w in ((0, 512), (512, 256)):
                        nc.tensor.matmul(
                            pv[:, lo:lo + w],
                            lhsT,
                            wvs[:, ct, lo:lo + w],
                            start=(ct == 0),
                            stop=(ct == CT - 1),
                        )
                vt = V[:, nt, :].rearrange("p (h e) -> p h e", e=D + 1)
                nc.vector.tensor_copy(
                    vt[:, :, 0:D], pv[:, 0:C].rearrange("p (h d) -> p h d", d=D)
                )

            # per-window filler schedule, paced to each window's PE
            # slack and the consumer deadlines (C(p) before window p,
            # V[kt] before PV(0,kt) in window 1)
            win_fillers = [
                [(c_unit, 1, 0), (c_unit, 1, 1)] + [(b_unit, nt) for nt in range(6)],
                [(b_unit, 6), (b_unit, 7), (c_unit, 2, 0), (c_unit, 2, 1)],
                [(c_unit, 3, 0), (c_unit, 3, 1)],
                [(c_unit, 4, 0), (c_unit, 4, 1)],
                [(c_unit, 5, 0), (c_unit, 5, 1)],
                [],
            ]

            def filler_slots(p):
                wf = win_fillers[p]
                return {(i * NT) // len(wf): f for i, f in enumerate(wf)} if wf else {}

            def emit_s(p, kt):
                """S^T for pair p, token tile kt; both halves row-tiled and
                emitted back-to-back per qc chunk so they stream
                concurrently. Returns [sps_h0, sps_h1]."""
                sps = [
                    psA.tile([128, N], F32, tag="ps", name=f"s{2 * p + i}_{kt}")
                    for i in range(2)
                ]
                prev_mm = None
                for qc in range(2):
                    for half in range(2):
                        po = half * 64
                        mm = nc.tensor.matmul(
                            sps[half][:, qc * 512:(qc + 1) * 512],
                            KT[po:po + 64, p, kt * 128:(kt + 1) * 128],
                            QT[po:po + 64, p, qc * 512:(qc + 1) * 512],
                            start=True,
                            stop=True,
                            tile_position=(po, 0),
                        )
                        if prev_mm is not None:
                            tile.add_dep_helper(
                                mm.ins, prev_mm.ins, sync=False,
                                reason="pin S-half interleave for co-stream",
                            )
                        prev_mm = mm
                return sps

            def emit_exp(p, kt, sps):
                row = []
                for half in range(2):
                    h = 2 * p + half
                    e = ep.tile([128, N], FP16, tag="e", name=f"e{h}_{kt}")
                    nc.scalar.activation(e[:], sps[half][:], EXP, scale=SCALE)
                    row.append(e)
                return row

            def emit_pv(pcps, pes, pp, kt):
                for half in range(2):
                    h = 2 * pp + half
                    for qc in range(2):
                        nc.tensor.matmul(
                            pcps[half][:, qc * 512:(qc + 1) * 512],
                            V[:, kt, h * (D + 1):(h + 1) * (D + 1)],
                            pes[kt][half][:, qc * 512:(qc + 1) * 512],
                            start=(kt == 0),
                            stop=(kt == NT - 1),
                        )

            def norm_chain(pcps, pp):
                """5-step evac/normalize pipeline for pair pp, emitted one
                step per kt slot so no long serial DVE chain ever sits in
                front of the filler evacs that release PE tiles."""
                dens, rcs, rcrs = [], [], []

                def s_evac():
                    for half in range(2):
                        po = half * 64
                        nc.vector.tensor_copy(
                            ctxN[po:po + 64, pp, :], pcps[half][0:D, :]
                        )

                def s_den():
                    for half in range(2):
                        den = normp.tile(
                            [1, N], F32, tag="den", name=f"dn{2 * pp + half}"
                        )
                        nc.vector.tensor_copy(den[:], pcps[half][D:D + 1, :])
                        dens.append(den)

                def s_rc():
                    for half in range(2):
                        rc = normp.tile(
                            [1, N], F32, tag="rc", name=f"rc{2 * pp + half}"
                        )
                        nc.vector.reciprocal_approx_fast(rc[:], dens[half][:])
                        rcs.append(rc)

                def s_rcr():
                    for half in range(2):
                        rcr = normp.tile(
                            [1, N], FP16, tag="rcr", name=f"rr{2 * pp + half}"
                        )
                        nc.vector.tensor_copy(rcr[:], rcs[half][:])
                        rcrs.append(rcr)

                def s_bc():
                    bc_ps = psA.tile([128, N], F32, tag="ps", name=f"bcp{pp}")
                    for qc in range(2):
                        for half in range(2):
                            nc.tensor.matmul(
                                bc_ps[:, qc * 512:(qc + 1) * 512],
                                ones_mask[half][:],
                                rcrs[half][:, qc * 512:(qc + 1) * 512],
                                start=(half == 0),
                                stop=(half == 1),
                            )
                    nc.vector.tensor_mul(ctxN[:, pp, :], ctxN[:, pp, :], bc_ps[:])

                return [s_evac, s_den, s_rc, s_rcr, s_bc]

            chains = []

            def pop_chain_step():
                if chains:
                    chains[0].pop(0)()
                    if not chains[0]:
                        chains.pop(0)

            # ---- prologue: Q/K for pair 0 ------------------------------
            c_unit(0, 0)
            c_unit(0, 1)

            # ---- attention windows -------------------------------------
            prev = None
            for p in range(CT):
                slots = filler_slots(p)
                cps = [
                    psC.tile([D + 1, N], F32, tag="ctx", name=f"ctx{2 * p + i}")
                    for i in range(2)
                ]
                es = []
                for kt in range(NT):
                    sps = emit_s(p, kt)
                    es.append(emit_exp(p, kt, sps))
                    if prev is not None:
                        emit_pv(prev[0], prev[1], prev[2], kt)
                    if 2 <= kt <= 6:
                        pop_chain_step()
                    f = slots.get(kt)
                    if f is not None:
                        f[0](*f[1:])
                if prev is not None:
                    chains.append(norm_chain(prev[0], prev[2]))
                prev = (cps, es, p)
            # ---- output projection units (ct=5 emitted last so partial
            # sums over ct 0..4 can run during the final pair's PV drain)
            e_tiles = {}

            def e_mms(nt, ps, cts, start, stop):
                for lo, w in ((0, 512), (512, 256)):
                    for i, ct in enumerate(cts):
                        nc.tensor.matmul(
                            ps[:, lo:lo + w],
                            ctxN[:, ct, nt * 128:(nt + 1) * 128],
                            wps[:, ct, lo:lo + w],
                            start=start and i == 0,
                            stop=stop and i == len(cts) - 1,
                        )

            def e_final(nt):
                # alternate PSUM pools: psC's ctx slots are free in the tail,
                # giving 4 accumulators in flight instead of 2
                if nt % 2 == 0:
                    ps = psA.tile([128, N], F32, tag="ps", name=f"po{nt}")
                else:
                    ps = psC.tile([128, N], F32, tag="ctx", name=f"po{nt}")
                e_mms(nt, ps, range(CT), True, True)
                ob = outp.tile([128, C], F32, tag="ob", name=f"ob{nt}")
                nc.vector.tensor_add(ob[:], ps[:, 0:C], bias_sb[:])
                nc.sync.dma_start(out_d[nt * 128:(nt + 1) * 128, :], ob[:])

            # drain: PV for the final pair (pair-4's norm chain spread over
            # it), then the final pair's norm, then the output projection
            for kt in range(NT):
                emit_pv(prev[0], prev[1], prev[2], kt)
                if kt >= 1:
                    pop_chain_step()
            chains.append(norm_chain(prev[0], prev[2]))
            while chains:
                pop_chain_step()
            for nt in range(NT):
                e_final(nt)

    nc.compile()
    return nc


_CACHE = {}


def _get_nc():
    if "nc" not in _CACHE:
        _CACHE["nc"] = build()
    return _CACHE["nc"]


def run(inputs, trace=False):
    """Run on hardware; returns (full output [8,1024,768] f32, results)."""
    nc = _get_nc()
    x = np.asarray(inputs["x"], dtype=np.float32)
    w_qkv = np.asarray(inputs["w_qkv"], dtype=np.float32)
    w_proj = np.asarray(inputs["w_proj"], dtype=np.float32)
    b_proj = np.asarray(inputs["b_proj"], dtype=np.float32)

    xT = np.ascontiguousarray(x.transpose(0, 2, 1)).astype(np.float16)
    wqT = w_qkv.T  # [C, 3C]
    # block-major Q/K weights: block 2*jt+base is the [C,128] feature slice,
    # stored as [128, 6*128] with partition p holding rows {p, p+128, ...} --
    # one contiguous fat-packet DMA delivers exactly one c_unit's weights
    wqk = np.empty((12 * 128, C), np.float16)
    for jt in range(6):
        for base in range(2):
            blk = wqT[:, base * C + jt * 128:base * C + (jt + 1) * 128]
            wqk[(2 * jt + base) * 128:(2 * jt + base + 1) * 128] = (
                blk.reshape(6, 128, 128).transpose(1, 0, 2).reshape(128, C)
            )
    wv = np.ascontiguousarray(wqT[:, 2 * C:3 * C]).astype(np.float16)
    wp = np.ascontiguousarray(w_proj.T).astype(np.float16)
    bias_bc = np.ascontiguousarray(np.broadcast_to(b_proj.reshape(1, C), (128, C)))

    ones_v = np.ones((128, H), dtype=np.float16)
    mask = np.kron(np.eye(2), np.ones((1, 64))).astype(np.float16)
    in_maps = [
        {
            "xT": xT[b],
            "wqk": wqk,
            "wv": wv,
            "wp": wp,
            "bias_bc": bias_bc,
            "ones_v": ones_v,
            "ones_mask": mask,
        }
        for b in range(N_CORES)
    ]
    res = run_bass_kernel_spmd(nc, in_maps, list(range(N_CORES)), trace=trace)
    out = np.stack([res.results[b]["out"] for b in range(N_CORES)])
    return out, res


def kernel(x, w_qkv, w_proj, b_proj):
    out, _ = run(
        {"x": x, "w_qkv": w_qkv, "w_proj": w_proj, "b_proj": b_proj}, trace=False
    )
    return out


# revision 31
# speedup vs baseline: 1.1700x; 1.1700x over previous
"""Multi-head self-attention (B=8, N=1024, C=768, H=12, D=64) on 8 Trainium2
NeuronCores, batch-parallel (one batch element per core).

v2: fully software-pipelined single-pass schedule. The exp stream (ACT) starts
as soon as Q/K for head-pair 0 exist (~10us in) and the V/QK/proj matmuls are
interleaved into the attention windows as PE filler work, instead of running
as serial phases with ACT idle.

Per-core dataflow (activations feature-major, "T" = [feature, token]):
  xT [768,1024] --(PE)--> QT,KT [768,1024] (d-major) and V [1024,768+ones]
  S^T[k,q] = KT_h^T x QT_h            (K=d=64; two heads of a pair via PE
                                       row-tiling at partitions 0/64,
                                       emitted back-to-back so the two
                                       halves stream concurrently)
  E = exp(S^T * scale) -> fp16        (ACT; no max-subtract: |S*scale| < 9)
  ctxU^T[d,q] (+den row) = V_ext_h^T x E   (ones column in V -> softmax den)
  normalize: recip(den) from PSUM, K=1 masked ones-matmul broadcast, one mul
  out[q,o] = ctxN^T x wpT + bias(bcast, DVE add)

Schedule: pair p's S/exp windows carry PV of pair p-1 (one-pair lag) plus a
filler unit per kt (remaining QK jt-slices, V token-tiles). All PSUM evacs run
on DVE; ACT does exp only. V-ones + matmul masks are built on-chip (memset)
instead of the 12k-packet strided DMA flood the old version had.
"""
import numpy as np

import concourse.bass as bass
import concourse.tile as tile
from concourse import bacc, mybir
from concourse.bass_utils import run_bass_kernel_spmd

N_CORES = 8
N = 1024          # tokens per core (batch element)
C = 768           # model dim
H = 12            # heads
D = 64            # head dim
SCALE = D ** -0.5
NT = N // 128     # 8 token tiles
CT = C // 128     # 6 feature tiles
F32 = mybir.dt.float32
F32R = mybir.dt.float32r
FP16 = mybir.dt.float16
EXP = mybir.ActivationFunctionType.Exp


def _r(ap):
    return ap.bitcast(F32R)


def build():
    nc = bacc.Bacc(
        "TRN2", target_bir_lowering=False, debug=False, num_devices=N_CORES
    )
    xT_d = nc.dram_tensor("xT", [C, N], FP16, kind="ExternalInput").ap()
    wqk_d = nc.dram_tensor("wqk", [12 * 128, C], FP16, kind="ExternalInput").ap()
    wv_d = nc.dram_tensor("wv", [C, C], FP16, kind="ExternalInput").ap()
    wp_d = nc.dram_tensor("wp", [C, C], FP16, kind="ExternalInput").ap()
    bias_d = nc.dram_tensor("bias_bc", [128, C], F32, kind="ExternalInput").ap()
    ones_d = nc.dram_tensor("ones_v", [128, H], FP16, kind="ExternalInput").ap()
    mask_d = nc.dram_tensor("ones_mask", [2, 128], FP16, kind="ExternalInput").ap()
    out_d = nc.dram_tensor("out", [N, C], F32, kind="ExternalOutput").ap()

    with tile.TileContext(nc) as tc:
        with (
            tc.tile_pool(name="big", bufs=1) as big,
            tc.tile_pool(name="e", bufs=18) as ep,
            tc.tile_pool(name="outb", bufs=2) as outp,
            tc.tile_pool(name="norm", bufs=4) as normp,
            tc.tile_pool(name="psA", bufs=2, space="PSUM") as psA,
            tc.tile_pool(name="psC", bufs=2, space="PSUM") as psC,
        ):
            # ---- persistent SBUF tensors -------------------------------
            xqk = big.tile([128, CT, N], FP16, name="xqk", tag="xqk")
            wqk = big.tile([128, 12, C], FP16, name="wqk", tag="wqk")
            wvs = big.tile([128, CT, C], FP16, name="wvs", tag="wvs")
            wps = big.tile([128, CT, C], FP16, name="wps", tag="wps")
            QT = big.tile([128, CT, N], FP16, name="QT", tag="QT")
            KT = big.tile([128, CT, N], FP16, name="KT", tag="KT")
            V = big.tile([128, NT, H * (D + 1)], FP16, name="V", tag="V")
            ctxN = big.tile([128, CT, N], FP16, name="ctxN", tag="ctxN")
            bias_sb = big.tile([128, C], F32, name="bias_sb", tag="bias")
            ones_sb = big.tile([128, H], FP16, name="ones_sb", tag="ones")
            ones_mask = [
                big.tile([1, 128], FP16, name=f"mask{i}", tag=f"mask{i}")
                for i in range(2)
            ]

            # ---- input DMAs (order = earliest-needed first) ------------
            # sync queue: per-ct (x, wqk) pairs so the head-pair-0 QK
            # projection streams behind the DMAs ct by ct; all contiguous
            for ct in range(CT):
                nc.sync.dma_start(xqk[:, ct, :], xT_d[ct * 128:(ct + 1) * 128, :])
            for b in range(12):
                nc.sync.dma_start(wqk[:, b, :], wqk_d[b * 128:(b + 1) * 128, :])
            # gpsimd queue: V weights, proj weights, constants (contiguous)
            nc.gpsimd.dma_start(ones_sb[:], ones_d[:])
            for i in range(2):
                nc.gpsimd.dma_start(ones_mask[i][:], mask_d[i:i + 1, :])
            for ct in range(CT):
                nc.gpsimd.dma_start(wvs[:, ct, :], wv_d[ct * 128:(ct + 1) * 128, :])
            for ct in range(CT):
                nc.gpsimd.dma_start(wps[:, ct, :], wp_d[ct * 128:(ct + 1) * 128, :])
            nc.gpsimd.dma_start(bias_sb[:], bias_d[:])

            # scatter the ones column into V on-chip (plain DVE copies)
            v4 = V[:].rearrange("p nt (h e) -> p nt h e", e=D + 1)
            for nt in range(NT):
                nc.vector.tensor_copy(
                    v4[:, nt, :, D:D + 1],
                    ones_sb[:].rearrange("p (h o) -> p h o", o=1),
                )


            # ---- work units --------------------------------------------
            def c_unit(jt, base):
                """QK projection slice jt (features jt*128..+127) for Q
                (base=0) or K (base=1); writes QT/KT[:, jt, :]."""
                ps = psA.tile([128, N], F32, tag="ps", name=f"c{base}_{jt}")
                for ct in range(CT):
                    lhsT = wqk[:, 2 * jt + base, ct * 128:(ct + 1) * 128]
                    for qc in range(2):
                        nc.tensor.matmul(
                            ps[:, qc * 512:(qc + 1) * 512],
                            lhsT,
                            xqk[:, ct, qc * 512:(qc + 1) * 512],
                            start=(ct == 0),
                            stop=(ct == CT - 1),
                        )
                dst = QT if base == 0 else KT
                nc.vector.tensor_copy(dst[:, jt, :], ps[:])

            def b_unit(nt):
                """V projection for token tile nt; writes V[:, nt, :]."""
                pv = psA.tile([128, N], F32, tag="ps", name=f"pv{nt}")
                for ct in range(CT):
                    lhsT = xqk[:, ct, nt * 128:(nt + 1) * 128]
                    for lo, w in ((0, 512), (512, 256)):
                        nc.tensor.matmul(
                            pv[:, lo:lo + w],
                            lhsT,
                            wvs[:, ct, lo:lo + w],
                            start=(ct == 0),
                            stop=(ct == CT - 1),
                        )
                vt = V[:, nt, :].rearrange("p (h e) -> p h e", e=D + 1)
                nc.vector.tensor_copy(
                    vt[:, :, 0:D], pv[:, 0:C].rearrange("p (h d) -> p h d", d=D)
                )

            # per-window filler schedule, paced to each window's PE
            # slack and the consumer deadlines (C(p) before window p,
            # V[kt] before PV(0,kt) in window 1)
            win_fillers = [
                [(c_unit, 1, 0), (c_unit, 1, 1)] + [(b_unit, nt) for nt in range(6)],
                [(b_unit, 6), (b_unit, 7), (c_unit, 2, 0), (c_unit, 2, 1)],
                [(c_unit, 3, 0), (c_unit, 3, 1)],
                [(c_unit, 4, 0), (c_unit, 4, 1)],
                [(c_unit, 5, 0), (c_unit, 5, 1)],
                [],
            ]

            def filler_slots(p):
                wf = win_fillers[p]
                return {(i * NT) // len(wf): f for i, f in enumerate(wf)} if wf else {}

            def emit_s(p, kt):
                """S^T for pair p, token tile kt; both halves row-tiled and
                emitted back-to-back per qc chunk so they stream
                concurrently. Returns [sps_h0, sps_h1]."""
                sps = [
                    psA.tile([128, N], F32, tag="ps", name=f"s{2 * p + i}_{kt}")
                    for i in range(2)
                ]
                for qc in range(2):
                    for half in range(2):
                        po = half * 64
                        nc.tensor.matmul(
                            sps[half][:, qc * 512:(qc + 1) * 512],
                            KT[po:po + 64, p, kt * 128:(kt + 1) * 128],
                            QT[po:po + 64, p, qc * 512:(qc + 1) * 512],
                            start=True,
                            stop=True,
                            tile_position=(po, 0),
                        )
                return sps

            def emit_exp(p, kt, sps):
                row = []
                for half in range(2):
                    h = 2 * p + half
                    e = ep.tile([128, N], FP16, tag="e", name=f"e{h}_{kt}")
                    nc.scalar.activation(e[:], sps[half][:], EXP, scale=SCALE)
                    row.append(e)
                return row

            def emit_pv(pcps, pes, pp, kt):
                for half in range(2):
                    h = 2 * pp + half
                    for qc in range(2):
                        nc.tensor.matmul(
                            pcps[half][:, qc * 512:(qc + 1) * 512],
                            V[:, kt, h * (D + 1):(h + 1) * (D + 1)],
                            pes[kt][half][:, qc * 512:(qc + 1) * 512],
                            start=(kt == 0),
                            stop=(kt == NT - 1),
                        )

            def norm_chain(pcps, pp):
                """5-step evac/normalize pipeline for pair pp, emitted one
                step per kt slot so no long serial DVE chain ever sits in
                front of the filler evacs that release PE tiles."""
                dens, rcs, rcrs = [], [], []

                def s_evac():
                    for half in range(2):
                        po = half * 64
                        nc.vector.tensor_copy(
                            ctxN[po:po + 64, pp, :], pcps[half][0:D, :]
                        )

                def s_den():
                    for half in range(2):
                        den = normp.tile(
                            [1, N], F32, tag="den", name=f"dn{2 * pp + half}"
                        )
                        nc.vector.tensor_copy(den[:], pcps[half][D:D + 1, :])
                        dens.append(den)

                def s_rc():
                    for half in range(2):
                        rc = normp.tile(
                            [1, N], F32, tag="rc", name=f"rc{2 * pp + half}"
                        )
                        nc.vector.reciprocal_approx_fast(rc[:], dens[half][:])
                        rcs.append(rc)

                def s_rcr():
                    for half in range(2):
                        rcr = normp.tile(
                            [1, N], FP16, tag="rcr", name=f"rr{2 * pp + half}"
                        )
                        nc.vector.tensor_copy(rcr[:], rcs[half][:])
                        rcrs.append(rcr)

                def s_bc():
                    bc_ps = psA.tile([128, N], F32, tag="ps", name=f"bcp{pp}")
                    for qc in range(2):
                        for half in range(2):
                            nc.tensor.matmul(
                                bc_ps[:, qc * 512:(qc + 1) * 512],
                                ones_mask[half][:],
                                rcrs[half][:, qc * 512:(qc + 1) * 512],
                                start=(half == 0),
                                stop=(half == 1),
                            )
                    nc.vector.tensor_mul(ctxN[:, pp, :], ctxN[:, pp, :], bc_ps[:])

                return [s_evac, s_den, s_rc, s_rcr, s_bc]

            chains = []

            def pop_chain_step():
                if chains:
                    chains[0].pop(0)()
                    if not chains[0]:
                        chains.pop(0)

            # ---- prologue: Q/K for pair 0 ------------------------------
            c_unit(0, 0)
            c_unit(0, 1)

            # ---- attention windows -------------------------------------
            prev = None
            for p in range(CT):
                slots = filler_slots(p)
                cps = [
                    psC.tile([D + 1, N], F32, tag="ctx", name=f"ctx{2 * p + i}")
                    for i in range(2)
                ]
                es = []
                for kt in range(NT):
                    sps = emit_s(p, kt)
                    es.append(emit_exp(p, kt, sps))
                    if prev is not None:
                        emit_pv(prev[0], prev[1], prev[2], kt)
                    if 2 <= kt <= 6:
                        pop_chain_step()
                    f = slots.get(kt)
                    if f is not None:
                        f[0](*f[1:])
                if prev is not None:
                    chains.append(norm_chain(prev[0], prev[2]))
                prev = (cps, es, p)
            # ---- output projection units (ct=5 emitted last so partial
            # sums over ct 0..4 can run during the final pair's PV drain)
            e_tiles = {}

            def e_mms(nt, ps, cts, start, stop):
                for lo, w in ((0, 512), (512, 256)):
                    for i, ct in enumerate(cts):
                        nc.tensor.matmul(
                            ps[:, lo:lo + w],
                            ctxN[:, ct, nt * 128:(nt + 1) * 128],
                            wps[:, ct, lo:lo + w],
                            start=start and i == 0,
                            stop=stop and i == len(cts) - 1,
                        )

            def e_final(nt):
                # alternate PSUM pools: psC's ctx slots are free in the tail,
                # giving 4 accumulators in flight instead of 2
                if nt % 2 == 0:
                    ps = psA.tile([128, N], F32, tag="ps", name=f"po{nt}")
                else:
                    ps = psC.tile([128, N], F32, tag="ctx", name=f"po{nt}")
                e_mms(nt, ps, range(CT), True, True)
                ob = outp.tile([128, C], F32, tag="ob", name=f"ob{nt}")
                nc.vector.tensor_add(ob[:], ps[:, 0:C], bias_sb[:])
                nc.sync.dma_start(out_d[nt * 128:(nt + 1) * 128, :], ob[:])

            # drain: PV for the final pair (pair-4's norm chain spread over
            # it), then the final pair's norm, then the output projection
            for kt in range(NT):
                emit_pv(prev[0], prev[1], prev[2], kt)
                if kt >= 1:
                    pop_chain_step()
            chains.append(norm_chain(prev[0], prev[2]))
            while chains:
                pop_chain_step()
            for nt in range(NT):
                e_final(nt)

    nc.compile()
    return nc


_CACHE = {}


def _get_nc():
    if "nc" not in _CACHE:
        _CACHE["nc"] = build()
    return _CACHE["nc"]


def run(inputs, trace=False):
    """Run on hardware; returns (full output [8,1024,768] f32, results)."""
    nc = _get_nc()
    x = np.asarray(inputs["x"], dtype=np.float32)
    w_qkv = np.asarray(inputs["w_qkv"], dtype=np.float32)
    w_proj = np.asarray(inputs["w_proj"], dtype=np.float32)
    b_proj = np.asarray(inputs["b_proj"], dtype=np.float32)

    xT = np.ascontiguousarray(x.transpose(0, 2, 1)).astype(np.float16)
    wqT = w_qkv.T  # [C, 3C]
    # block-major Q/K weights: block 2*jt+base is the [C,128] feature slice,
    # stored as [128, 6*128] with partition p holding rows {p, p+128, ...} --
    # one contiguous fat-packet DMA delivers exactly one c_unit's weights
    wqk = np.empty((12 * 128, C), np.float16)
    for jt in range(6):
        for base in range(2):
            blk = wqT[:, base * C + jt * 128:base * C + (jt + 1) * 128]
            wqk[(2 * jt + base) * 128:(2 * jt + base + 1) * 128] = (
                blk.reshape(6, 128, 128).transpose(1, 0, 2).reshape(128, C)
            )
    wv = np.ascontiguousarray(wqT[:, 2 * C:3 * C]).astype(np.float16)
    wp = np.ascontiguousarray(w_proj.T).astype(np.float16)
    bias_bc = np.ascontiguousarray(np.broadcast_to(b_proj.reshape(1, C), (128, C)))

    ones_v = np.ones((128, H), dtype=np.float16)
    mask = np.kron(np.eye(2), np.ones((1, 64))).astype(np.float16)
    in_maps = [
        {
            "xT": xT[b],
            "wqk": wqk,
            "wv": wv,
            "wp": wp,
            "bias_bc": bias_bc,
            "ones_v": ones_v,
            "ones_mask": mask,
        }
        for b in range(N_CORES)
    ]
    res = run_bass_kernel_spmd(nc, in_maps, list(range(N_CORES)), trace=trace)
    out = np.stack([res.results[b]["out"] for b in range(N_CORES)])
    return out, res


def kernel(x, w_qkv, w_proj, b_proj):
    out, _ = run(
        {"x": x, "w_qkv": w_qkv, "w_proj": w_proj, "b_proj": b_proj}, trace=False
    )
    return out


# revision 32
# speedup vs baseline: 1.4327x; 1.2245x over previous
"""Multi-head self-attention (B=8, N=1024, C=768, H=12, D=64) on 8 Trainium2
NeuronCores, batch-parallel (one batch element per core).

v2: fully software-pipelined single-pass schedule. The exp stream (ACT) starts
as soon as Q/K for head-pair 0 exist (~10us in) and the V/QK/proj matmuls are
interleaved into the attention windows as PE filler work, instead of running
as serial phases with ACT idle.

Per-core dataflow (activations feature-major, "T" = [feature, token]):
  xT [768,1024] --(PE)--> QT,KT [768,1024] (d-major) and V [1024,768+ones]
  S^T[k,q] = KT_h^T x QT_h            (K=d=64; two heads of a pair via PE
                                       row-tiling at partitions 0/64,
                                       emitted back-to-back so the two
                                       halves stream concurrently)
  E = exp(S^T * scale) -> fp16        (ACT; no max-subtract: |S*scale| < 9)
  ctxU^T[d,q] (+den row) = V_ext_h^T x E   (ones column in V -> softmax den)
  normalize: recip(den) from PSUM, K=1 masked ones-matmul broadcast, one mul
  out[q,o] = ctxN^T x wpT + bias(bcast, DVE add)

Schedule: pair p's S/exp windows carry PV of pair p-1 (one-pair lag) plus a
filler unit per kt (remaining QK jt-slices, V token-tiles). All PSUM evacs run
on DVE; ACT does exp only. V-ones + matmul masks are built on-chip (memset)
instead of the 12k-packet strided DMA flood the old version had.
"""
import numpy as np

import concourse.bass as bass
import concourse.tile as tile
from concourse import bacc, mybir
from concourse.bass_utils import run_bass_kernel_spmd

N_CORES = 8
N = 1024          # tokens per core (batch element)
C = 768           # model dim
H = 12            # heads
D = 64            # head dim
SCALE = D ** -0.5
NT = N // 128     # 8 token tiles
CT = C // 128     # 6 feature tiles
F32 = mybir.dt.float32
F32R = mybir.dt.float32r
FP16 = mybir.dt.float16
EXP = mybir.ActivationFunctionType.Exp


def _r(ap):
    return ap.bitcast(F32R)


def build():
    nc = bacc.Bacc(
        "TRN2", target_bir_lowering=False, debug=False, num_devices=N_CORES
    )
    xT_d = nc.dram_tensor("xT", [C, N], FP16, kind="ExternalInput").ap()
    wqk_d = nc.dram_tensor("wqk", [12 * 128, C], FP16, kind="ExternalInput").ap()
    wv_d = nc.dram_tensor("wv", [C, C], FP16, kind="ExternalInput").ap()
    wp_d = nc.dram_tensor("wp", [C, C], FP16, kind="ExternalInput").ap()
    bias_d = nc.dram_tensor("bias_bc", [128, C], F32, kind="ExternalInput").ap()
    ones_d = nc.dram_tensor("ones_v", [128, H], FP16, kind="ExternalInput").ap()
    mask_d = nc.dram_tensor("ones_mask", [2, 128], FP16, kind="ExternalInput").ap()
    out_d = nc.dram_tensor("out", [N, C], F32, kind="ExternalOutput").ap()

    with tile.TileContext(nc) as tc:
        with (
            tc.tile_pool(name="big", bufs=1) as big,
            tc.tile_pool(name="e", bufs=18) as ep,
            tc.tile_pool(name="outb", bufs=2) as outp,
            tc.tile_pool(name="norm", bufs=4) as normp,
            tc.tile_pool(name="psA", bufs=2, space="PSUM") as psA,
            tc.tile_pool(name="psC", bufs=2, space="PSUM") as psC,
        ):
            # ---- persistent SBUF tensors -------------------------------
            xqk = big.tile([128, CT, N], FP16, name="xqk", tag="xqk")
            wqk = big.tile([128, 12, C], FP16, name="wqk", tag="wqk")
            wvs = big.tile([128, CT, C], FP16, name="wvs", tag="wvs")
            wps = big.tile([128, CT, C], FP16, name="wps", tag="wps")
            QT = big.tile([128, CT, N], FP16, name="QT", tag="QT")
            KT = big.tile([128, CT, N], FP16, name="KT", tag="KT")
            V = big.tile([128, NT, H * (D + 1)], FP16, name="V", tag="V")
            ctxN = big.tile([128, CT, N], FP16, name="ctxN", tag="ctxN")
            bias_sb = big.tile([128, C], F32, name="bias_sb", tag="bias")
            ones_sb = big.tile([128, H], FP16, name="ones_sb", tag="ones")
            ones_mask = [
                big.tile([1, 128], FP16, name=f"mask{i}", tag=f"mask{i}")
                for i in range(2)
            ]

            # ---- input DMAs (order = earliest-needed first) ------------
            # sync queue: per-ct (x, wqk) pairs so the head-pair-0 QK
            # projection streams behind the DMAs ct by ct; all contiguous
            def _dma_x(ct):
                nc.sync.dma_start(xqk[:, ct, :], xT_d[ct * 128:(ct + 1) * 128, :])

            def _dma_w(b):
                nc.sync.dma_start(wqk[:, b, :], wqk_d[b * 128:(b + 1) * 128, :])

            _dma_x(0)
            _dma_w(0)
            _dma_w(1)
            for ct in range(1, CT):
                _dma_x(ct)
            for b in range(2, 12):
                _dma_w(b)
            # gpsimd queue: V weights, proj weights, constants (contiguous)
            nc.gpsimd.dma_start(ones_sb[:], ones_d[:])
            for i in range(2):
                nc.gpsimd.dma_start(ones_mask[i][:], mask_d[i:i + 1, :])
            for ct in range(CT):
                nc.gpsimd.dma_start(wvs[:, ct, :], wv_d[ct * 128:(ct + 1) * 128, :])
            for ct in range(CT):
                nc.gpsimd.dma_start(wps[:, ct, :], wp_d[ct * 128:(ct + 1) * 128, :])
            nc.gpsimd.dma_start(bias_sb[:], bias_d[:])

            # scatter the ones column into V on-chip (plain DVE copies)
            v4 = V[:].rearrange("p nt (h e) -> p nt h e", e=D + 1)
            for nt in range(NT):
                nc.vector.tensor_copy(
                    v4[:, nt, :, D:D + 1],
                    ones_sb[:].rearrange("p (h o) -> p h o", o=1),
                )


            # ---- work units --------------------------------------------
            def c_unit(jt, base):
                """QK projection slice jt (features jt*128..+127) for Q
                (base=0) or K (base=1); writes QT/KT[:, jt, :]."""
                ps = psA.tile([128, N], F32, tag="ps", name=f"c{base}_{jt}")
                for ct in range(CT):
                    lhsT = wqk[:, 2 * jt + base, ct * 128:(ct + 1) * 128]
                    for qc in range(2):
                        nc.tensor.matmul(
                            ps[:, qc * 512:(qc + 1) * 512],
                            lhsT,
                            xqk[:, ct, qc * 512:(qc + 1) * 512],
                            start=(ct == 0),
                            stop=(ct == CT - 1),
                        )
                dst = QT if base == 0 else KT
                nc.vector.tensor_copy(dst[:, jt, :], ps[:])

            def b_unit(nt):
                """V projection for token tile nt; writes V[:, nt, :]."""
                pv = psA.tile([128, N], F32, tag="ps", name=f"pv{nt}")
                for ct in range(CT):
                    lhsT = xqk[:, ct, nt * 128:(nt + 1) * 128]
                    for lo, w in ((0, 512), (512, 256)):
                        nc.tensor.matmul(
                            pv[:, lo:lo + w],
                            lhsT,
                            wvs[:, ct, lo:lo + w],
                            start=(ct == 0),
                            stop=(ct == CT - 1),
                        )
                vt = V[:, nt, :].rearrange("p (h e) -> p h e", e=D + 1)
                nc.vector.tensor_copy(
                    vt[:, :, 0:D], pv[:, 0:C].rearrange("p (h d) -> p h d", d=D)
                )

            # per-window filler schedule, paced to each window's PE
            # slack and the consumer deadlines (C(p) before window p,
            # V[kt] before PV(0,kt) in window 1)
            win_fillers = [
                [(c_unit, 1, 0), (c_unit, 1, 1)] + [(b_unit, nt) for nt in range(6)],
                [(b_unit, 6), (b_unit, 7), (c_unit, 2, 0), (c_unit, 2, 1)],
                [(c_unit, 3, 0), (c_unit, 3, 1)],
                [(c_unit, 4, 0), (c_unit, 4, 1)],
                [(c_unit, 5, 0), (c_unit, 5, 1)],
                [],
            ]

            def filler_slots(p):
                wf = win_fillers[p]
                return {(i * NT) // len(wf): f for i, f in enumerate(wf)} if wf else {}

            def emit_s(p, kt):
                """S^T for pair p, token tile kt; both halves row-tiled and
                emitted back-to-back per qc chunk so they stream
                concurrently. Returns [sps_h0, sps_h1]."""
                sps = [
                    psA.tile([128, N], F32, tag="ps", name=f"s{2 * p + i}_{kt}")
                    for i in range(2)
                ]
                for qc in range(2):
                    for half in range(2):
                        po = half * 64
                        nc.tensor.matmul(
                            sps[half][:, qc * 512:(qc + 1) * 512],
                            KT[po:po + 64, p, kt * 128:(kt + 1) * 128],
                            QT[po:po + 64, p, qc * 512:(qc + 1) * 512],
                            start=True,
                            stop=True,
                            tile_position=(po, 0),
                        )
                return sps

            def emit_exp(p, kt, sps):
                row = []
                for half in range(2):
                    h = 2 * p + half
                    e = ep.tile([128, N], FP16, tag="e", name=f"e{h}_{kt}")
                    nc.scalar.activation(e[:], sps[half][:], EXP, scale=SCALE)
                    row.append(e)
                return row

            def emit_pv(pcps, pes, pp, kt):
                for half in range(2):
                    h = 2 * pp + half
                    for qc in range(2):
                        nc.tensor.matmul(
                            pcps[half][:, qc * 512:(qc + 1) * 512],
                            V[:, kt, h * (D + 1):(h + 1) * (D + 1)],
                            pes[kt][half][:, qc * 512:(qc + 1) * 512],
                            start=(kt == 0),
                            stop=(kt == NT - 1),
                        )

            def norm_chain(pcps, pp):
                """5-step evac/normalize pipeline for pair pp, emitted one
                step per kt slot so no long serial DVE chain ever sits in
                front of the filler evacs that release PE tiles."""
                dens, rcs, rcrs = [], [], []

                def s_evac():
                    for half in range(2):
                        po = half * 64
                        nc.vector.tensor_copy(
                            ctxN[po:po + 64, pp, :], pcps[half][0:D, :]
                        )

                def s_den():
                    for half in range(2):
                        den = normp.tile(
                            [1, N], F32, tag="den", name=f"dn{2 * pp + half}"
                        )
                        nc.vector.tensor_copy(den[:], pcps[half][D:D + 1, :])
                        dens.append(den)

                def s_rc():
                    for half in range(2):
                        rc = normp.tile(
                            [1, N], F32, tag="rc", name=f"rc{2 * pp + half}"
                        )
                        nc.vector.reciprocal_approx_fast(rc[:], dens[half][:])
                        rcs.append(rc)

                def s_rcr():
                    for half in range(2):
                        rcr = normp.tile(
                            [1, N], FP16, tag="rcr", name=f"rr{2 * pp + half}"
                        )
                        nc.vector.tensor_copy(rcr[:], rcs[half][:])
                        rcrs.append(rcr)

                def s_bc():
                    bc_ps = psA.tile([128, N], F32, tag="ps", name=f"bcp{pp}")
                    for qc in range(2):
                        for half in range(2):
                            nc.tensor.matmul(
                                bc_ps[:, qc * 512:(qc + 1) * 512],
                                ones_mask[half][:],
                                rcrs[half][:, qc * 512:(qc + 1) * 512],
                                start=(half == 0),
                                stop=(half == 1),
                            )
                    nc.vector.tensor_mul(ctxN[:, pp, :], ctxN[:, pp, :], bc_ps[:])

                return [s_evac, s_den, s_rc, s_rcr, s_bc]

            chains = []

            def pop_chain_step():
                if chains:
                    chains[0].pop(0)()
                    if not chains[0]:
                        chains.pop(0)

            # ---- prologue: Q/K for pair 0 ------------------------------
            c_unit(0, 0)
            c_unit(0, 1)

            # ---- attention windows -------------------------------------
            prev = None
            for p in range(CT):
                slots = filler_slots(p)
                cps = [
                    psC.tile([D + 1, N], F32, tag="ctx", name=f"ctx{2 * p + i}")
                    for i in range(2)
                ]
                es = []
                for kt in range(NT):
                    sps = emit_s(p, kt)
                    es.append(emit_exp(p, kt, sps))
                    if prev is not None:
                        emit_pv(prev[0], prev[1], prev[2], kt)
                    if 2 <= kt <= 6:
                        pop_chain_step()
                    f = slots.get(kt)
                    if f is not None:
                        f[0](*f[1:])
                if prev is not None:
                    chains.append(norm_chain(prev[0], prev[2]))
                prev = (cps, es, p)
            # ---- output projection units (ct=5 emitted last so partial
            # sums over ct 0..4 can run during the final pair's PV drain)
            e_tiles = {}

            def e_mms(nt, ps, cts, start, stop):
                for lo, w in ((0, 512), (512, 256)):
                    for i, ct in enumerate(cts):
                        nc.tensor.matmul(
                            ps[:, lo:lo + w],
                            ctxN[:, ct, nt * 128:(nt + 1) * 128],
                            wps[:, ct, lo:lo + w],
                            start=start and i == 0,
                            stop=stop and i == len(cts) - 1,
                        )

            def e_final(nt):
                # alternate PSUM pools: psC's ctx slots are free in the tail,
                # giving 4 accumulators in flight instead of 2
                if nt % 2 == 0:
                    ps = psA.tile([128, N], F32, tag="ps", name=f"po{nt}")
                else:
                    ps = psC.tile([128, N], F32, tag="ctx", name=f"po{nt}")
                e_mms(nt, ps, range(CT), True, True)
                ob = outp.tile([128, C], F32, tag="ob", name=f"ob{nt}")
                nc.vector.tensor_add(ob[:], ps[:, 0:C], bias_sb[:])
                nc.sync.dma_start(out_d[nt * 128:(nt + 1) * 128, :], ob[:])

            # drain: PV for the final pair (pair-4's norm chain spread over
            # it), then the final pair's norm, then the output projection
            for kt in range(NT):
                emit_pv(prev[0], prev[1], prev[2], kt)
                if kt >= 1:
                    pop_chain_step()
            ch5 = norm_chain(prev[0], prev[2])
            ch5[0]()  # evac
            ch5[1]()  # den copies (release psC slots)
            # two projection partials on the freed psC slots overlap the
            # remaining reciprocal/broadcast latency
            part = {}
            for nt in (0, 1):
                ps = psC.tile([128, N], F32, tag="ctx", name=f"po{nt}")
                e_mms(nt, ps, range(CT - 1), True, False)
                part[nt] = ps
            ch5[2]()  # rc
            ch5[3]()  # rcr
            ch5[4]()  # bc + mul
            for nt in (0, 1):
                ps = part[nt]
                e_mms(nt, ps, [CT - 1], False, True)
                ob = outp.tile([128, C], F32, tag="ob", name=f"ob{nt}")
                nc.vector.tensor_add(ob[:], ps[:, 0:C], bias_sb[:])
                nc.sync.dma_start(out_d[nt * 128:(nt + 1) * 128, :], ob[:])
            for nt in range(2, NT):
                e_final(nt)

    nc.compile()
    return nc


_CACHE = {}


def _get_nc():
    if "nc" not in _CACHE:
        _CACHE["nc"] = build()
    return _CACHE["nc"]


def run(inputs, trace=False):
    """Run on hardware; returns (full output [8,1024,768] f32, results)."""
    nc = _get_nc()
    x = np.asarray(inputs["x"], dtype=np.float32)
    w_qkv = np.asarray(inputs["w_qkv"], dtype=np.float32)
    w_proj = np.asarray(inputs["w_proj"], dtype=np.float32)
    b_proj = np.asarray(inputs["b_proj"], dtype=np.float32)

    xT = np.ascontiguousarray(x.transpose(0, 2, 1)).astype(np.float16)
    wqT = w_qkv.T  # [C, 3C]
    # block-major Q/K weights: block 2*jt+base is the [C,128] feature slice,
    # stored as [128, 6*128] with partition p holding rows {p, p+128, ...} --
    # one contiguous fat-packet DMA delivers exactly one c_unit's weights
    wqk = np.empty((12 * 128, C), np.float16)
    for jt in range(6):
        for base in range(2):
            blk = wqT[:, base * C + jt * 128:base * C + (jt + 1) * 128]
            wqk[(2 * jt + base) * 128:(2 * jt + base + 1) * 128] = (
                blk.reshape(6, 128, 128).transpose(1, 0, 2).reshape(128, C)
            )
    wv = np.ascontiguousarray(wqT[:, 2 * C:3 * C]).astype(np.float16)
    wp = np.ascontiguousarray(w_proj.T).astype(np.float16)
    bias_bc = np.ascontiguousarray(np.broadcast_to(b_proj.reshape(1, C), (128, C)))

    ones_v = np.ones((128, H), dtype=np.float16)
    mask = np.kron(np.eye(2), np.ones((1, 64))).astype(np.float16)
    in_maps = [
        {
            "xT": xT[b],
            "wqk": wqk,
            "wv": wv,
            "wp": wp,
            "bias_bc": bias_bc,
            "ones_v": ones_v,
            "ones_mask": mask,
        }
        for b in range(N_CORES)
    ]
    res = run_bass_kernel_spmd(nc, in_maps, list(range(N_CORES)), trace=trace)
    out = np.stack([res.results[b]["out"] for b in range(N_CORES)])
    return out, res


def kernel(x, w_qkv, w_proj, b_proj):
    out, _ = run(
        {"x": x, "w_qkv": w_qkv, "w_proj": w_proj, "b_proj": b_proj}, trace=False
    )
    return out
